# revision 75
# baseline (speedup 1.0000x reference)
"""CQAttention Trainium2 kernel.

Math (per batch b):
  S = (C*w3) @ Q^T + (C@w1)[:,None] + (Q@w2)[None,:] (+bias, dropped: softmax-invariant)
  Sq = softmax over q of qmask-masked S ; Sc = softmax over c of cmask-masked S
  A = Sq@Q ; Bm = Sq @ (Sc^T @ C) ; out = [C | A | C*A | C*Bm]

Device algorithm (no max-subtraction: |S| is small so exp is safe; masks become
additive -1e30 terms). All PE operands are bf16 (fp32 PSUM accumulate); the
host pre-packs the bf16 views so no on-chip casts are needed:
  CTb  = Cb^T (PE transposes of host-cast bf16 C)
  QT3w = [(Q^T)*w3 | w1 dup]          [d, 130] bf16  (host-prepared)
  ST   = QT3w[:, :128] @ CTb          [q, c]   (PE)
  E_q  = exp(ST + (rq + qneg)[q])     [q, c]   bf16  (rq+qneg host-fused)
  S2_k = CTb_k^T @ QT3w               [c, 130] = [S^T tile | rc dup]
  E2_k = exp(S2 + (rc + cneg)[c])     [c, q]   bf16  (rc from S2 col 128)
  t1   = sum_k E2_k^T @ [C|1]_k       [q, d+2] == unnormalized Sc^T C | colsum
  T1s  = [t1 * 1/colsum | 1]          [q, d+2] bf16
  psB  = E_q^T @ T1s                  [c, d+2] unnormalized Bm | rowsum
  psA  = E_q^T @ Q                    [c, d]   unnormalized A
  rr = 1/rowsum ; A = psA*rr ; CA = C*A ; CBm = C*psB*rr

Sharding: data-parallel over batch, 4 batches per core on 8 cores.
"""

import numpy as np

NEG_INF = -1e30
B_FULL, LC, LQ, D = 32, 1024, 128, 256
N_CORES = 8
NB = B_FULL // N_CORES  # batches per core
KC = LC // 128  # c-tiles per batch (8)

_CACHE = {}


def _build_nc():
    import concourse.bacc as bacc
    import concourse.mybir as mybir
    from concourse import tile
    from concourse.masks import make_identity

    fp32 = mybir.dt.float32
    bf16 = mybir.dt.bfloat16
    MULT = mybir.AluOpType.mult
    EXP = mybir.ActivationFunctionType.Exp
    IDENT = mybir.ActivationFunctionType.Identity

    nc = bacc.Bacc("TRN2", target_bir_lowering=False, debug=False)

    # bundle (bf16, per partition): Cb [KC*258] | QT3w [2*130] | Qb [258]
    #                               | rqq [1] | cneg [KC]
    NBND = KC * (D + 2) + 2 * 130 + (D + 2) + 1 + KC
    bnd_d = nc.dram_tensor("bnd", [NB, 128, NBND], bf16, kind="ExternalInput")
    out_d = nc.dram_tensor("out", [NB, LC, 4 * D], fp32, kind="ExternalOutput")

    with tile.TileContext(nc) as tc:
        with (
            tc.tile_pool(name="const", bufs=1) as const,
            tc.tile_pool(name="cpool", bufs=NB) as p_c,
            tc.tile_pool(name="cbpool", bufs=NB) as p_cb,
            tc.tile_pool(name="qpool", bufs=NB) as p_q,
            tc.tile_pool(name="mpool", bufs=NB) as p_m,
            tc.tile_pool(name="ctpool", bufs=2) as p_ct,
            tc.tile_pool(name="epool", bufs=2) as p_e,
            tc.tile_pool(name="opool", bufs=4) as p_o,
            tc.tile_pool(name="smpool", bufs=4) as p_sm,
            tc.tile_pool(name="pspt", bufs=1, space="PSUM") as ps_pt,
            tc.tile_pool(name="psst", bufs=1, space="PSUM") as ps_st,
            tc.tile_pool(name="pss2", bufs=2, space="PSUM") as ps_s2,
            tc.tile_pool(name="pst1", bufs=1, space="PSUM") as ps_t1,
            tc.tile_pool(name="psacc", bufs=3, space="PSUM") as ps_acc,
        ):
            identb = const.tile([128, 128], bf16)
            make_identity(nc, identb)

            # ---- hoisted input loads: one bf16 bundle per batch on the
            # Scalar HWDGE ring (early; frees the Sync ring for stores) ----
            O_CB = 0
            O_QT = KC * (D + 2)
            O_QB = O_QT + 2 * 130
            O_RQ = O_QB + (D + 2)
            O_CN = O_RQ + 1
            C1s, Cb1s, Qb1s, QT3ws, rqqs, cnegs = [], [], [], [], [], []
            for b in range(NB):
                bnd = p_cb.tile([128, NBND], bf16, tag="bnd")
                ldq = nc.sync if b % 2 == 0 else nc.scalar
                if b == 0:
                    # split across both rings so the first transposes and
                    # the q-side operands land as early as possible
                    half = O_QT // 2
                    nc.sync.dma_start(bnd[:, 0:half], bnd_d.ap()[b, :, 0:half])
                    nc.scalar.dma_start(
                        bnd[:, half:NBND], bnd_d.ap()[b, :, half:NBND]
                    )
                else:
                    ldq.dma_start(bnd, bnd_d.ap()[b])
                Cb1s.append(
                    bnd[:, O_CB:O_QT].rearrange("p (k d) -> p k d", d=D + 2)
                )
                QT3ws.append(bnd[:, O_QT:O_QB].rearrange("p (t d) -> p t d", d=130))
                Qb1s.append(bnd[:, O_QB : O_QB + D + 2])
                cnegs.append(bnd[:, O_CN : O_CN + KC])
                # fp32 copy of the exp bias (plays safe with ACT bias dtype)
                rqq = p_m.tile([128, 1], fp32, tag="rqq")
                nc.vector.tensor_copy(rqq, bnd[:, O_RQ : O_RQ + 1])
                rqqs.append(rqq)

            E_qs, T1ss, C1x, osbx, rrx = [], [], {}, {}, {}

            def head_stages(b):
                """Head of batch b (incl. the T1s-independent psA chain)."""
                Cb1, Qb1, QT3w = Cb1s[b], Qb1s[b], QT3ws[b]
                rqq, cneg = rqqs[b], cnegs[b]
                CTb = p_ct.tile([128, 2, LC], bf16, tag="ct", name=f"CTb{b}")
                E_q = p_e.tile([128, LC], bf16, tag="eq", name=f"Eq{b}")
                E_qs.append(E_q)
                E2 = p_e.tile([128, KC, 128], bf16, tag="e2", name=f"E2{b}")
                C1 = p_c.tile([128, KC, D], fp32, tag="c", name=f"C1_{b}")
                C1x[b] = C1

                def cast_store():
                    nc.vector.tensor_copy(C1[:, 0:4], Cb1[:, 0:4, 0:D])
                    nc.vector.tensor_copy(C1[:, 4:8], Cb1[:, 4:8, 0:D])
                    # halves on separate rings, each gated only on its cast
                    for h in range(2):
                        eng = nc.sync if (b + h) % 2 == 0 else nc.gpsimd
                        eng.dma_start(
                            out_d.ap()[
                                b, h * 512 : (h + 1) * 512, 0:D
                            ].rearrange("(k p) d -> p k d", p=128),
                            C1[:, 4 * h : 4 * h + 4],
                        )

                def tile_a(k):
                    kk = k % 4
                    if kk == 0:
                        osbx[(b, k // 4)] = p_o.tile(
                            [128, 4, 3 * D], fp32, tag="osb", name=f"osb{b}_{k}"
                        )
                    osb = osbx[(b, k // 4)]
                    eq_k = E_q[:, k * 128 : (k + 1) * 128]
                    # psA = Eq^T @ [Q|1]: rowsum in col D, independent of T1s
                    psA = ps_acc.tile(
                        [128, D + 2], fp32, tag="acc", name=f"psA{b}_{k}"
                    )
                    nc.tensor.matmul(psA, eq_k, Qb1, start=True, stop=True)
                    rr = p_sm.tile(
                        [128, 1], fp32, tag="rr", name=f"rr{b}_{k}", bufs=18
                    )
                    rrx[(b, k)] = rr
                    nc.vector.reciprocal(rr, psA[:, D : D + 1])
                    # A = psA * rr  (per-partition scale; 4 of 8 on DVE)
                    if k in (1, 3, 5, 7):
                        nc.vector.tensor_scalar_mul(osb[:, kk, 0:D], psA[:, 0:D], rr)
                    else:
                        nc.scalar.mul(osb[:, kk, 0:D], psA[:, 0:D], rr)

                def ct_group(g):
                    dk, h = g // 2, g % 2
                    pt = ps_pt.tile([128, 512], bf16, tag="pt", name=f"pt{b}_{g}")
                    for j in range(4):
                        k = h * 4 + j
                        nc.tensor.transpose(
                            pt[:, j * 128 : (j + 1) * 128],
                            Cb1[:, k, dk * 128 : (dk + 1) * 128],
                            identb,
                        )
                    nc.vector.tensor_copy(CTb[:, dk, h * 512 : (h + 1) * 512], pt)

                def st_half(h):
                    st = ps_st.tile([128, 512], fp32, tag="st", name=f"st{b}_{h}")
                    for dk in range(2):
                        nc.tensor.matmul(
                            st,
                            QT3w[:, dk, 0:128],
                            CTb[:, dk, h * 512 : (h + 1) * 512],
                            start=(dk == 0),
                            stop=(dk == 1),
                        )
                    nc.scalar.activation(
                        E_q[:, h * 512 : (h + 1) * 512], st, EXP, bias=rqq
                    )

                def s2_pair(kp):
                    for k in (2 * kp, 2 * kp + 1):
                        s2 = ps_s2.tile(
                            [128, 130], fp32, tag="s2", name=f"s2_{b}_{k}"
                        )
                        for dk in range(2):
                            nc.tensor.matmul(
                                s2,
                                CTb[:, dk, k * 128 : (k + 1) * 128],
                                QT3w[:, dk],
                                start=(dk == 0),
                                stop=(dk == 1),
                            )
                        bias_k = p_sm.tile(
                            [128, 1], fp32, tag="biask", name=f"bk{b}_{k}"
                        )
                        nc.scalar.activation(
                            bias_k, s2[:, 128:129], IDENT, bias=cneg[:, k : k + 1]
                        )
                        nc.scalar.activation(E2[:, k], s2[:, 0:128], EXP, bias=bias_k)

                t1_box = {}

                def t1_acc(half):
                    if half == 0:
                        t1_box["t1"] = ps_t1.tile(
                            [128, D + 2], fp32, tag="t1", name=f"t1_{b}"
                        )
                    t1 = t1_box["t1"]
                    for k in range(4 * half, 4 * half + 4):
                        nc.tensor.matmul(
                            t1,
                            E2[:, k],
                            Cb1[:, k],
                            start=(k == 0),
                            stop=(k == KC - 1),
                        )
                    if half == 1:
                        recipT = p_sm.tile(
                            [128, 1], fp32, tag="recipT", name=f"rT{b}"
                        )
                        nc.vector.reciprocal(recipT, t1[:, D : D + 1])
                        T1s = p_sm.tile([128, D], bf16, tag="t1s", name=f"T1s{b}")
                        nc.vector.tensor_scalar_mul(T1s, t1[:, 0:D], recipT)
                        T1ss.append(T1s)

                return [
                    lambda: ct_group(0),
                    lambda: (ct_group(1), cast_store()),
                    lambda: ct_group(2),
                    lambda: ct_group(3),
                    lambda: st_half(0),
                    lambda: s2_pair(0),
                    lambda: (st_half(1), s2_pair(1)),
                    lambda: (s2_pair(2), tile_a(0), tile_a(1)),
                    lambda: (s2_pair(3), t1_acc(0), tile_a(2), tile_a(3)),
                    lambda: (t1_acc(1), tile_a(4), tile_a(5)),
                    lambda: (tile_a(6), tile_a(7)),
                ]

            def tail_stages(b):
                """T1s-dependent part of batch b's tail: psB / CBm / CA."""
                E_q = E_qs[b]
                C1 = C1x[b]
                Bm = p_sm.tile(
                    [128, 2, D], fp32, tag="bm", name=f"Bm{b}", bufs=2
                )

                def tile_b(k):
                    T1s = T1ss[b]
                    kk = k % 4
                    osb = osbx[(b, k // 4)]
                    eq_k = E_q[:, k * 128 : (k + 1) * 128]
                    rr = rrx[(b, k)]
                    psB = ps_acc.tile(
                        [128, D + 2], fp32, tag="acc", name=f"psB{b}_{k}"
                    )
                    nc.tensor.matmul(psB[:, 0:D], eq_k, T1s, start=True, stop=True)
                    if k in (2, 3):
                        # offload 2 of 8 CBm tiles: Bm on ACT, C-mul on GPS
                        nc.scalar.mul(Bm[:, k - 2], psB[:, 0:D], rr)
                        if k == 3:
                            nc.gpsimd.tensor_mul(
                                osb[:, 2:4, 2 * D : 3 * D], C1[:, 2:4], Bm
                            )
                    else:
                        # CBm = (psB * rr) * C  (DVE fused)
                        nc.vector.scalar_tensor_tensor(
                            osb[:, kk, 2 * D : 3 * D], psB[:, 0:D], rr, C1[:, k],
                            MULT, MULT,
                        )
                    if kk == 3:
                        # CA = C * A for 4 tiles in one GPSIMD op
                        nc.gpsimd.tensor_mul(
                            osb[:, :, D : 2 * D],
                            C1[:, k - 3 : k + 1],
                            osb[:, :, 0:D],
                        )
                        if b == NB - 1:
                            # last batch: 2-tile stores on both rings to
                            # shrink the final DMA drain
                            for half in range(2):
                                eng = nc.sync if half == 0 else nc.scalar
                                k0 = k - 3 + 2 * half
                                eng.dma_start(
                                    out_d.ap()[
                                        b, k0 * 128 : (k0 + 2) * 128, D : 4 * D
                                    ].rearrange("(k p) n -> p k n", p=128),
                                    osb[:, 2 * half : 2 * half + 2],
                                )
                        else:
                            # alternate: Sync HWDGE ring / GPSIMD SWDGE ring
                            eng = (
                                nc.sync if (2 * b + k // 4) % 2 == 0 else nc.gpsimd
                            )
                            eng.dma_start(
                                out_d.ap()[
                                    b, (k - 3) * 128 : (k + 1) * 128, D : 4 * D
                                ].rearrange("(k p) n -> p k n", p=128),
                                osb,
                            )

                return [(lambda kk_: lambda: tile_b(kk_))(k) for k in range(KC)]

            # fine-grained software pipelining: interleave head(b) stages
            # with tail(b-1) stages so no engine queue head-of-line blocks
            for step in range(NB + 1):
                hs = head_stages(step) if step < NB else []
                ts = tail_stages(step - 1) if step >= 1 else []
                n = max(len(hs), len(ts))
                for i in range(n):
                    if i < len(hs):
                        hs[i]()
                    if i < len(ts):
                        ts[i]()

    nc.compile()
    return nc


def _get_nc():
    if "nc" not in _CACHE:
        _CACHE["nc"] = _build_nc()
    return _CACHE["nc"]


def _make_in_maps(C, Q, cmask, qmask, Wo_w):
    import ml_dtypes

    bf16 = ml_dtypes.bfloat16
    C = np.ascontiguousarray(C, dtype=np.float32)
    Q = np.ascontiguousarray(Q, dtype=np.float32)
    Wo_w = Wo_w.astype(np.float32)
    w1, w2, w3 = Wo_w[:D], Wo_w[D : 2 * D], Wo_w[2 * D :]

    NBND = KC * (D + 2) + 2 * 130 + (D + 2) + 1 + KC
    O_QT = KC * (D + 2)
    O_QB = O_QT + 2 * 130
    O_RQ = O_QB + (D + 2)
    O_CN = O_RQ + 1
    bnd = np.empty((B_FULL, 128, NBND), dtype=bf16)

    # Cb: tile layout with ones columns
    cb = bnd[:, :, 0:O_QT].reshape(B_FULL, 128, KC, D + 2)
    cb[:, :, :, 0:D] = C.reshape(B_FULL, KC, 128, D).transpose(0, 2, 1, 3)
    cb[:, :, :, D:] = 1.0

    # QT3w: [p, dk, j<128] = Q[b,j,dk*128+p]*w3[dk*128+p]; cols 128:130 = w1
    qt3 = bnd[:, :, O_QT:O_QB].reshape(B_FULL, 128, 2, 130)
    qt = Q.transpose(0, 2, 1).reshape(B_FULL, 2, 128, 128).transpose(0, 2, 1, 3)
    qt3[:, :, :, 0:128] = qt * w3.reshape(2, 128).T[None, :, :, None]
    qt3[:, :, :, 128:130] = w1.reshape(2, 128).T[None, :, :, None]

    # Qb with ones columns (rowsum source for psA)
    bnd[:, :, O_QB : O_QB + D] = Q
    bnd[:, :, O_QB + D : O_RQ] = 1.0

    # rq + qneg fused exp bias
    bnd[:, :, O_RQ] = Q @ w2 + (1.0 - qmask.astype(np.float32)) * NEG_INF

    # cneg in c-tile layout
    cneg = (1.0 - cmask.astype(np.float32)) * NEG_INF
    bnd[:, :, O_CN:] = cneg.reshape(B_FULL, KC, 128).transpose(0, 2, 1)

    in_maps = []
    for i in range(N_CORES):
        sl = slice(i * NB, (i + 1) * NB)
        in_maps.append({"bnd": np.ascontiguousarray(bnd[sl])})
    return in_maps


def kernel(C, Q, cmask, qmask, Wo_w, Wo_b):
    from concourse.bass_utils import run_bass_kernel_spmd

    nc = _get_nc()
    in_maps = _make_in_maps(C, Q, cmask, qmask, Wo_w)
    res = run_bass_kernel_spmd(nc, in_maps, core_ids=list(range(N_CORES)))
    out = np.concatenate([res.results[i]["out"] for i in range(N_CORES)], axis=0)
    return out


# revision 77
# speedup vs baseline: 1.0764x; 1.0764x over previous
"""CQAttention Trainium2 kernel.

Math (per batch b):
  S = (C*w3) @ Q^T + (C@w1)[:,None] + (Q@w2)[None,:] (+bias, dropped: softmax-invariant)
  Sq = softmax over q of qmask-masked S ; Sc = softmax over c of cmask-masked S
  A = Sq@Q ; Bm = Sq @ (Sc^T @ C) ; out = [C | A | C*A | C*Bm]

Device algorithm (no max-subtraction: |S| is small so exp is safe; masks become
additive -1e30 terms). All PE operands are bf16 (fp32 PSUM accumulate); the
host pre-packs the bf16 views so no on-chip casts are needed:
  CTb  = Cb^T (PE transposes of host-cast bf16 C)
  QT3w = [(Q^T)*w3 | w1 dup]          [d, 130] bf16  (host-prepared)
  ST   = QT3w[:, :128] @ CTb          [q, c]   (PE)
  E_q  = exp(ST + (rq + qneg)[q])     [q, c]   bf16  (rq+qneg host-fused)
  S2_k = CTb_k^T @ QT3w               [c, 130] = [S^T tile | rc dup]
  E2_k = exp(S2 + (rc + cneg)[c])     [c, q]   bf16  (rc from S2 col 128)
  t1   = sum_k E2_k^T @ [C|1]_k       [q, d+2] == unnormalized Sc^T C | colsum
  T1s  = [t1 * 1/colsum | 1]          [q, d+2] bf16
  psB  = E_q^T @ T1s                  [c, d+2] unnormalized Bm | rowsum
  psA  = E_q^T @ Q                    [c, d]   unnormalized A
  rr = 1/rowsum ; A = psA*rr ; CA = C*A ; CBm = C*psB*rr

Sharding: data-parallel over batch, 4 batches per core on 8 cores.
"""

import numpy as np

NEG_INF = -1e30
B_FULL, LC, LQ, D = 32, 1024, 128, 256
N_CORES = 8
NB = B_FULL // N_CORES  # batches per core
KC = LC // 128  # c-tiles per batch (8)

_CACHE = {}


def _build_nc():
    import concourse.bacc as bacc
    import concourse.mybir as mybir
    from concourse import tile
    from concourse.masks import make_identity

    fp32 = mybir.dt.float32
    bf16 = mybir.dt.bfloat16
    MULT = mybir.AluOpType.mult
    EXP = mybir.ActivationFunctionType.Exp
    IDENT = mybir.ActivationFunctionType.Identity

    nc = bacc.Bacc("TRN2", target_bir_lowering=False, debug=False)

    # bundle (bf16, per partition): Cb [KC*258] | QT3w [2*130] | Qb [258]
    #                               | rqq [1] | cneg [KC]
    NBND = KC * (D + 2) + 2 * 130 + (D + 2) + 1 + KC
    bnd_d = nc.dram_tensor("bnd", [NB, 128, NBND], bf16, kind="ExternalInput")
    out_d = nc.dram_tensor("out", [NB, LC, 4 * D], fp32, kind="ExternalOutput")

    with tile.TileContext(nc) as tc:
        with (
            tc.tile_pool(name="const", bufs=1) as const,
            tc.tile_pool(name="cpool", bufs=NB) as p_c,
            tc.tile_pool(name="cbpool", bufs=NB) as p_cb,
            tc.tile_pool(name="qpool", bufs=NB) as p_q,
            tc.tile_pool(name="mpool", bufs=NB) as p_m,
            tc.tile_pool(name="ctpool", bufs=2) as p_ct,
            tc.tile_pool(name="epool", bufs=2) as p_e,
            tc.tile_pool(name="opool", bufs=4) as p_o,
            tc.tile_pool(name="smpool", bufs=4) as p_sm,
            tc.tile_pool(name="pspt", bufs=1, space="PSUM") as ps_pt,
            tc.tile_pool(name="psst", bufs=1, space="PSUM") as ps_st,
            tc.tile_pool(name="pss2", bufs=2, space="PSUM") as ps_s2,
            tc.tile_pool(name="pst1", bufs=1, space="PSUM") as ps_t1,
            tc.tile_pool(name="psacc", bufs=3, space="PSUM") as ps_acc,
        ):
            identb = const.tile([128, 128], bf16)
            make_identity(nc, identb)

            # ---- hoisted input loads: one bf16 bundle per batch on the
            # Scalar HWDGE ring (early; frees the Sync ring for stores) ----
            O_CB = 0
            O_QT = KC * (D + 2)
            O_QB = O_QT + 2 * 130
            O_RQ = O_QB + (D + 2)
            O_CN = O_RQ + 1
            C1s, Cb1s, Qb1s, QT3ws, rqqs, cnegs = [], [], [], [], [], []
            for b in range(NB):
                bnd = p_cb.tile([128, NBND], bf16, tag="bnd")
                ldq = nc.sync if b % 2 == 0 else nc.scalar
                if b == 0:
                    # split across both rings so the first transposes and
                    # the q-side operands land as early as possible
                    half = O_QT // 2
                    nc.sync.dma_start(bnd[:, 0:half], bnd_d.ap()[b, :, 0:half])
                    nc.scalar.dma_start(
                        bnd[:, half:NBND], bnd_d.ap()[b, :, half:NBND]
                    )
                else:
                    ldq.dma_start(bnd, bnd_d.ap()[b])
                Cb1s.append(
                    bnd[:, O_CB:O_QT].rearrange("p (k d) -> p k d", d=D + 2)
                )
                QT3ws.append(bnd[:, O_QT:O_QB].rearrange("p (t d) -> p t d", d=130))
                Qb1s.append(bnd[:, O_QB : O_QB + D + 2])
                cnegs.append(bnd[:, O_CN : O_CN + KC])
                # fp32 copy of the exp bias (plays safe with ACT bias dtype)
                rqq = p_m.tile([128, 1], fp32, tag="rqq")
                nc.vector.tensor_copy(rqq, bnd[:, O_RQ : O_RQ + 1])
                rqqs.append(rqq)

            E_qs, T1ss, C1x, osbx, rrx = [], [], {}, {}, {}

            def head_stages(b):
                """Head of batch b (incl. the T1s-independent psA chain)."""
                Cb1, Qb1, QT3w = Cb1s[b], Qb1s[b], QT3ws[b]
                rqq, cneg = rqqs[b], cnegs[b]
                CTb = p_ct.tile([128, 2, LC], bf16, tag="ct", name=f"CTb{b}")
                E_q = p_e.tile([128, LC], bf16, tag="eq", name=f"Eq{b}")
                E_qs.append(E_q)
                E2 = p_e.tile([128, KC, 128], bf16, tag="e2", name=f"E2{b}")
                C1 = p_c.tile([128, KC, D], fp32, tag="c", name=f"C1_{b}")
                C1x[b] = C1

                def cast_store():
                    nc.vector.tensor_copy(C1[:, 0:4], Cb1[:, 0:4, 0:D])
                    nc.vector.tensor_copy(C1[:, 4:8], Cb1[:, 4:8, 0:D])
                    # halves on separate rings, each gated only on its cast
                    for h in range(2):
                        eng = nc.sync if (b + h) % 2 == 0 else nc.gpsimd
                        eng.dma_start(
                            out_d.ap()[
                                b, h * 512 : (h + 1) * 512, 0:D
                            ].rearrange("(k p) d -> p k d", p=128),
                            C1[:, 4 * h : 4 * h + 4],
                        )

                def tile_a(k):
                    kk = k % 4
                    if kk == 0:
                        osbx[(b, k // 4)] = p_o.tile(
                            [128, 4, 3 * D], fp32, tag="osb", name=f"osb{b}_{k}"
                        )
                    osb = osbx[(b, k // 4)]
                    eq_k = E_q[:, k * 128 : (k + 1) * 128]
                    # psA = Eq^T @ [Q|1]: rowsum in col D, independent of T1s
                    psA = ps_acc.tile(
                        [128, D + 2], fp32, tag="acc", name=f"psA{b}_{k}"
                    )
                    nc.tensor.matmul(psA, eq_k, Qb1, start=True, stop=True)
                    rr = p_sm.tile(
                        [128, 1], fp32, tag="rr", name=f"rr{b}_{k}", bufs=18
                    )
                    rrx[(b, k)] = rr
                    nc.vector.reciprocal(rr, psA[:, D : D + 1])
                    # A = psA * rr  (per-partition scale; 2 of 8 on DVE)
                    if k in (1, 5):
                        nc.vector.tensor_scalar_mul(osb[:, kk, 0:D], psA[:, 0:D], rr)
                    else:
                        nc.scalar.mul(osb[:, kk, 0:D], psA[:, 0:D], rr)

                def ct_group(g):
                    dk, h = g // 2, g % 2
                    pt = ps_pt.tile([128, 512], bf16, tag="pt", name=f"pt{b}_{g}")
                    for j in range(4):
                        k = h * 4 + j
                        nc.tensor.transpose(
                            pt[:, j * 128 : (j + 1) * 128],
                            Cb1[:, k, dk * 128 : (dk + 1) * 128],
                            identb,
                        )
                    nc.vector.tensor_copy(CTb[:, dk, h * 512 : (h + 1) * 512], pt)

                def st_half(h):
                    st = ps_st.tile([128, 512], fp32, tag="st", name=f"st{b}_{h}")
                    for dk in range(2):
                        nc.tensor.matmul(
                            st,
                            QT3w[:, dk, 0:128],
                            CTb[:, dk, h * 512 : (h + 1) * 512],
                            start=(dk == 0),
                            stop=(dk == 1),
                        )
                    nc.scalar.activation(
                        E_q[:, h * 512 : (h + 1) * 512], st, EXP, bias=rqq
                    )

                def s2_pair(kp):
                    for k in (2 * kp, 2 * kp + 1):
                        s2 = ps_s2.tile(
                            [128, 130], fp32, tag="s2", name=f"s2_{b}_{k}"
                        )
                        for dk in range(2):
                            nc.tensor.matmul(
                                s2,
                                CTb[:, dk, k * 128 : (k + 1) * 128],
                                QT3w[:, dk],
                                start=(dk == 0),
                                stop=(dk == 1),
                            )
                        bias_k = p_sm.tile(
                            [128, 1], fp32, tag="biask", name=f"bk{b}_{k}"
                        )
                        nc.vector.tensor_add(
                            bias_k, s2[:, 128:129], cneg[:, k : k + 1]
                        )
                        nc.scalar.activation(E2[:, k], s2[:, 0:128], EXP, bias=bias_k)

                t1_box = {}

                def t1_acc(half):
                    if half == 0:
                        t1_box["t1"] = ps_t1.tile(
                            [128, D + 2], fp32, tag="t1", name=f"t1_{b}"
                        )
                    t1 = t1_box["t1"]
                    for k in range(4 * half, 4 * half + 4):
                        nc.tensor.matmul(
                            t1,
                            E2[:, k],
                            Cb1[:, k],
                            start=(k == 0),
                            stop=(k == KC - 1),
                        )
                    if half == 1:
                        recipT = p_sm.tile(
                            [128, 1], fp32, tag="recipT", name=f"rT{b}"
                        )
                        nc.vector.reciprocal(recipT, t1[:, D : D + 1])
                        T1s = p_sm.tile([128, D], bf16, tag="t1s", name=f"T1s{b}")
                        nc.vector.tensor_scalar_mul(T1s, t1[:, 0:D], recipT)
                        T1ss.append(T1s)

                return [
                    lambda: ct_group(0),
                    lambda: (ct_group(1), cast_store()),
                    lambda: ct_group(2),
                    lambda: ct_group(3),
                    lambda: st_half(0),
                    lambda: s2_pair(0),
                    lambda: (st_half(1), s2_pair(1)),
                    lambda: (s2_pair(2), tile_a(0), tile_a(1)),
                    lambda: (s2_pair(3), t1_acc(0), tile_a(2), tile_a(3)),
                    lambda: (t1_acc(1), tile_a(4), tile_a(5)),
                    lambda: (tile_a(6), tile_a(7)),
                ]

            def tail_stages(b):
                """T1s-dependent part of batch b's tail: psB / CBm / CA."""
                E_q = E_qs[b]
                C1 = C1x[b]
                Bm = p_sm.tile(
                    [128, 2, D], fp32, tag="bm", name=f"Bm{b}", bufs=2
                )

                def tile_b(k):
                    T1s = T1ss[b]
                    kk = k % 4
                    osb = osbx[(b, k // 4)]
                    eq_k = E_q[:, k * 128 : (k + 1) * 128]
                    rr = rrx[(b, k)]
                    psB = ps_acc.tile(
                        [128, D + 2], fp32, tag="acc", name=f"psB{b}_{k}"
                    )
                    nc.tensor.matmul(psB[:, 0:D], eq_k, T1s, start=True, stop=True)
                    if k in (2, 3):
                        # offload 2 of 8 CBm tiles: Bm on ACT, C-mul on GPS
                        nc.scalar.mul(Bm[:, k - 2], psB[:, 0:D], rr)
                        if k == 3:
                            nc.gpsimd.tensor_mul(
                                osb[:, 2:4, 2 * D : 3 * D], C1[:, 2:4], Bm
                            )
                    else:
                        # CBm = (psB * rr) * C  (DVE fused)
                        nc.vector.scalar_tensor_tensor(
                            osb[:, kk, 2 * D : 3 * D], psB[:, 0:D], rr, C1[:, k],
                            MULT, MULT,
                        )
                    if kk == 3:
                        # CA = C * A for 4 tiles in one GPSIMD op
                        nc.gpsimd.tensor_mul(
                            osb[:, :, D : 2 * D],
                            C1[:, k - 3 : k + 1],
                            osb[:, :, 0:D],
                        )
                        if b == NB - 1:
                            # last batch: 2-tile stores on both rings to
                            # shrink the final DMA drain
                            for half in range(2):
                                eng = nc.sync if half == 0 else nc.scalar
                                k0 = k - 3 + 2 * half
                                eng.dma_start(
                                    out_d.ap()[
                                        b, k0 * 128 : (k0 + 2) * 128, D : 4 * D
                                    ].rearrange("(k p) n -> p k n", p=128),
                                    osb[:, 2 * half : 2 * half + 2],
                                )
                        else:
                            # alternate: Sync HWDGE ring / GPSIMD SWDGE ring
                            eng = (
                                nc.sync if (2 * b + k // 4) % 2 == 0 else nc.gpsimd
                            )
                            eng.dma_start(
                                out_d.ap()[
                                    b, (k - 3) * 128 : (k + 1) * 128, D : 4 * D
                                ].rearrange("(k p) n -> p k n", p=128),
                                osb,
                            )

                return [(lambda kk_: lambda: tile_b(kk_))(k) for k in range(KC)]

            # fine-grained software pipelining: interleave head(b) stages
            # with tail(b-1) stages so no engine queue head-of-line blocks
            for step in range(NB + 1):
                hs = head_stages(step) if step < NB else []
                ts = tail_stages(step - 1) if step >= 1 else []
                n = max(len(hs), len(ts))
                for i in range(n):
                    if i < len(hs):
                        hs[i]()
                    if i < len(ts):
                        ts[i]()

    nc.compile()
    return nc


def _get_nc():
    if "nc" not in _CACHE:
        _CACHE["nc"] = _build_nc()
    return _CACHE["nc"]


def _make_in_maps(C, Q, cmask, qmask, Wo_w):
    import ml_dtypes

    bf16 = ml_dtypes.bfloat16
    C = np.ascontiguousarray(C, dtype=np.float32)
    Q = np.ascontiguousarray(Q, dtype=np.float32)
    Wo_w = Wo_w.astype(np.float32)
    w1, w2, w3 = Wo_w[:D], Wo_w[D : 2 * D], Wo_w[2 * D :]

    NBND = KC * (D + 2) + 2 * 130 + (D + 2) + 1 + KC
    O_QT = KC * (D + 2)
    O_QB = O_QT + 2 * 130
    O_RQ = O_QB + (D + 2)
    O_CN = O_RQ + 1
    bnd = np.empty((B_FULL, 128, NBND), dtype=bf16)

    # Cb: tile layout with ones columns
    cb = bnd[:, :, 0:O_QT].reshape(B_FULL, 128, KC, D + 2)
    cb[:, :, :, 0:D] = C.reshape(B_FULL, KC, 128, D).transpose(0, 2, 1, 3)
    cb[:, :, :, D:] = 1.0

    # QT3w: [p, dk, j<128] = Q[b,j,dk*128+p]*w3[dk*128+p]; cols 128:130 = w1
    qt3 = bnd[:, :, O_QT:O_QB].reshape(B_FULL, 128, 2, 130)
    qt = Q.transpose(0, 2, 1).reshape(B_FULL, 2, 128, 128).transpose(0, 2, 1, 3)
    qt3[:, :, :, 0:128] = qt * w3.reshape(2, 128).T[None, :, :, None]
    qt3[:, :, :, 128:130] = w1.reshape(2, 128).T[None, :, :, None]

    # Qb with ones columns (rowsum source for psA)
    bnd[:, :, O_QB : O_QB + D] = Q
    bnd[:, :, O_QB + D : O_RQ] = 1.0

    # rq + qneg fused exp bias
    bnd[:, :, O_RQ] = Q @ w2 + (1.0 - qmask.astype(np.float32)) * NEG_INF

    # cneg in c-tile layout
    cneg = (1.0 - cmask.astype(np.float32)) * NEG_INF
    bnd[:, :, O_CN:] = cneg.reshape(B_FULL, KC, 128).transpose(0, 2, 1)

    in_maps = []
    for i in range(N_CORES):
        sl = slice(i * NB, (i + 1) * NB)
        in_maps.append({"bnd": np.ascontiguousarray(bnd[sl])})
    return in_maps


def kernel(C, Q, cmask, qmask, Wo_w, Wo_b):
    from concourse.bass_utils import run_bass_kernel_spmd

    nc = _get_nc()
    in_maps = _make_in_maps(C, Q, cmask, qmask, Wo_w)
    res = run_bass_kernel_spmd(nc, in_maps, core_ids=list(range(N_CORES)))
    out = np.concatenate([res.results[i]["out"] for i in range(N_CORES)], axis=0)
    return out


# revision 78
# speedup vs baseline: 1.0945x; 1.0168x over previous
"""CQAttention Trainium2 kernel.

Math (per batch b):
  S = (C*w3) @ Q^T + (C@w1)[:,None] + (Q@w2)[None,:] (+bias, dropped: softmax-invariant)
  Sq = softmax over q of qmask-masked S ; Sc = softmax over c of cmask-masked S
  A = Sq@Q ; Bm = Sq @ (Sc^T @ C) ; out = [C | A | C*A | C*Bm]

Device algorithm (no max-subtraction: |S| is small so exp is safe; masks become
additive -1e30 terms). All PE operands are bf16 (fp32 PSUM accumulate); the
host pre-packs the bf16 views so no on-chip casts are needed:
  CTb  = Cb^T (PE transposes of host-cast bf16 C)
  QT3w = [(Q^T)*w3 | w1 dup]          [d, 130] bf16  (host-prepared)
  ST   = QT3w[:, :128] @ CTb          [q, c]   (PE)
  E_q  = exp(ST + (rq + qneg)[q])     [q, c]   bf16  (rq+qneg host-fused)
  S2_k = CTb_k^T @ QT3w               [c, 130] = [S^T tile | rc dup]
  E2_k = exp(S2 + (rc + cneg)[c])     [c, q]   bf16  (rc from S2 col 128)
  t1   = sum_k E2_k^T @ [C|1]_k       [q, d+2] == unnormalized Sc^T C | colsum
  T1s  = [t1 * 1/colsum | 1]          [q, d+2] bf16
  psB  = E_q^T @ T1s                  [c, d+2] unnormalized Bm | rowsum
  psA  = E_q^T @ Q                    [c, d]   unnormalized A
  rr = 1/rowsum ; A = psA*rr ; CA = C*A ; CBm = C*psB*rr

Sharding: data-parallel over batch, 4 batches per core on 8 cores.
"""

import numpy as np

NEG_INF = -1e30
B_FULL, LC, LQ, D = 32, 1024, 128, 256
N_CORES = 8
NB = B_FULL // N_CORES  # batches per core
KC = LC // 128  # c-tiles per batch (8)

_CACHE = {}


def _build_nc():
    import concourse.bacc as bacc
    import concourse.mybir as mybir
    from concourse import tile
    from concourse.masks import make_identity

    fp32 = mybir.dt.float32
    bf16 = mybir.dt.bfloat16
    MULT = mybir.AluOpType.mult
    EXP = mybir.ActivationFunctionType.Exp

    nc = bacc.Bacc("TRN2", target_bir_lowering=False, debug=False)

    # bundle (bf16, per partition): Cb [KC*258] | QT3w [2*130] | Qb [258]
    #                               | rqq [1] | cneg [KC]
    NBND = KC * (D + 2) + 2 * 130 + (D + 2) + 1 + KC
    bnd_d = nc.dram_tensor("bnd", [NB, 128, NBND], bf16, kind="ExternalInput")
    out_d = nc.dram_tensor("out", [NB, LC, 4 * D], fp32, kind="ExternalOutput")

    with tile.TileContext(nc) as tc:
        with (
            tc.tile_pool(name="const", bufs=1) as const,
            tc.tile_pool(name="cpool", bufs=NB) as p_c,
            tc.tile_pool(name="cbpool", bufs=NB) as p_cb,
            tc.tile_pool(name="qpool", bufs=NB) as p_q,
            tc.tile_pool(name="mpool", bufs=NB) as p_m,
            tc.tile_pool(name="ctpool", bufs=2) as p_ct,
            tc.tile_pool(name="epool", bufs=2) as p_e,
            tc.tile_pool(name="opool", bufs=4) as p_o,
            tc.tile_pool(name="smpool", bufs=4) as p_sm,
            tc.tile_pool(name="pspt", bufs=1, space="PSUM") as ps_pt,
            tc.tile_pool(name="psst", bufs=1, space="PSUM") as ps_st,
            tc.tile_pool(name="pss2", bufs=2, space="PSUM") as ps_s2,
            tc.tile_pool(name="pst1", bufs=1, space="PSUM") as ps_t1,
            tc.tile_pool(name="psacc", bufs=3, space="PSUM") as ps_acc,
        ):
            identb = const.tile([128, 128], bf16)
            make_identity(nc, identb)

            # ---- hoisted input loads: one bf16 bundle per batch on the
            # Scalar HWDGE ring (early; frees the Sync ring for stores) ----
            O_CB = 0
            O_QT = KC * (D + 2)
            O_QB = O_QT + 2 * 130
            O_RQ = O_QB + (D + 2)
            O_CN = O_RQ + 1
            C1s, Cb1s, Qb1s, QT3ws, rqqs, cnegs = [], [], [], [], [], []
            for b in range(NB):
                bnd = p_cb.tile([128, NBND], bf16, tag="bnd")
                ldq = nc.sync if b % 2 == 0 else nc.scalar
                if b == 0:
                    # split across both rings so the first transposes and
                    # the q-side operands land as early as possible
                    half = O_QT // 2
                    nc.sync.dma_start(bnd[:, 0:half], bnd_d.ap()[b, :, 0:half])
                    nc.scalar.dma_start(
                        bnd[:, half:NBND], bnd_d.ap()[b, :, half:NBND]
                    )
                else:
                    ldq.dma_start(bnd, bnd_d.ap()[b])
                Cb1s.append(
                    bnd[:, O_CB:O_QT].rearrange("p (k d) -> p k d", d=D + 2)
                )
                QT3ws.append(bnd[:, O_QT:O_QB].rearrange("p (t d) -> p t d", d=130))
                Qb1s.append(bnd[:, O_QB : O_QB + D + 2])
                cnegs.append(bnd[:, O_CN : O_CN + KC])
                # fp32 copy of the exp bias (plays safe with ACT bias dtype)
                rqq = p_m.tile([128, 1], fp32, tag="rqq")
                nc.vector.tensor_copy(rqq, bnd[:, O_RQ : O_RQ + 1])
                rqqs.append(rqq)

            E_qs, T1ss, C1x, osbx, rrx = [], [], {}, {}, {}

            def head_stages(b):
                """Head of batch b (incl. the T1s-independent psA chain)."""
                Cb1, Qb1, QT3w = Cb1s[b], Qb1s[b], QT3ws[b]
                rqq, cneg = rqqs[b], cnegs[b]
                CTb = p_ct.tile([128, 2, LC], bf16, tag="ct", name=f"CTb{b}")
                E_q = p_e.tile([128, LC], bf16, tag="eq", name=f"Eq{b}")
                E_qs.append(E_q)
                E2 = p_e.tile([128, KC, 128], bf16, tag="e2", name=f"E2{b}")
                C1 = p_c.tile([128, KC, D], fp32, tag="c", name=f"C1_{b}")
                C1x[b] = C1

                def cast_store():
                    nc.vector.tensor_copy(C1[:, 0:4], Cb1[:, 0:4, 0:D])
                    nc.vector.tensor_copy(C1[:, 4:8], Cb1[:, 4:8, 0:D])
                    # halves on separate rings, each gated only on its cast
                    for h in range(2):
                        eng = nc.sync if (b + h) % 2 == 0 else nc.gpsimd
                        eng.dma_start(
                            out_d.ap()[
                                b, h * 512 : (h + 1) * 512, 0:D
                            ].rearrange("(k p) d -> p k d", p=128),
                            C1[:, 4 * h : 4 * h + 4],
                        )

                def tile_a(k):
                    kk = k % 4
                    if kk == 0:
                        osbx[(b, k // 4)] = p_o.tile(
                            [128, 4, 3 * D], fp32, tag="osb", name=f"osb{b}_{k}"
                        )
                    osb = osbx[(b, k // 4)]
                    eq_k = E_q[:, k * 128 : (k + 1) * 128]
                    # psA = Eq^T @ [Q|1]: rowsum in col D, independent of T1s
                    psA = ps_acc.tile(
                        [128, D + 2], fp32, tag="acc", name=f"psA{b}_{k}"
                    )
                    nc.tensor.matmul(psA, eq_k, Qb1, start=True, stop=True)
                    rr = p_sm.tile(
                        [128, 1], fp32, tag="rr", name=f"rr{b}_{k}", bufs=18
                    )
                    rrx[(b, k)] = rr
                    nc.vector.reciprocal(rr, psA[:, D : D + 1])
                    # A = psA * rr  (per-partition scale; 2 of 8 on DVE)
                    if k in (1, 5):
                        nc.vector.tensor_scalar_mul(osb[:, kk, 0:D], psA[:, 0:D], rr)
                    else:
                        nc.scalar.mul(osb[:, kk, 0:D], psA[:, 0:D], rr)

                def ct_group(g):
                    dk, h = g // 2, g % 2
                    pt = ps_pt.tile([128, 512], bf16, tag="pt", name=f"pt{b}_{g}")
                    for j in range(4):
                        k = h * 4 + j
                        nc.tensor.transpose(
                            pt[:, j * 128 : (j + 1) * 128],
                            Cb1[:, k, dk * 128 : (dk + 1) * 128],
                            identb,
                        )
                    nc.vector.tensor_copy(CTb[:, dk, h * 512 : (h + 1) * 512], pt)

                def st_half(h):
                    st = ps_st.tile([128, 512], fp32, tag="st", name=f"st{b}_{h}")
                    for dk in range(2):
                        nc.tensor.matmul(
                            st,
                            QT3w[:, dk, 0:128],
                            CTb[:, dk, h * 512 : (h + 1) * 512],
                            start=(dk == 0),
                            stop=(dk == 1),
                        )
                    nc.scalar.activation(
                        E_q[:, h * 512 : (h + 1) * 512], st, EXP, bias=rqq
                    )

                def s2_pair(kp):
                    for k in (2 * kp, 2 * kp + 1):
                        s2 = ps_s2.tile(
                            [128, 130], fp32, tag="s2", name=f"s2_{b}_{k}"
                        )
                        for dk in range(2):
                            nc.tensor.matmul(
                                s2,
                                CTb[:, dk, k * 128 : (k + 1) * 128],
                                QT3w[:, dk],
                                start=(dk == 0),
                                stop=(dk == 1),
                            )
                        bias_k = p_sm.tile(
                            [128, 1], fp32, tag="biask", name=f"bk{b}_{k}"
                        )
                        nc.vector.tensor_add(
                            bias_k, s2[:, 128:129], cneg[:, k : k + 1]
                        )
                        nc.scalar.activation(E2[:, k], s2[:, 0:128], EXP, bias=bias_k)

                t1_box = {}

                def t1_acc(half):
                    if half == 0:
                        t1_box["t1"] = ps_t1.tile(
                            [128, D + 2], fp32, tag="t1", name=f"t1_{b}"
                        )
                    t1 = t1_box["t1"]
                    for k in range(4 * half, 4 * half + 4):
                        nc.tensor.matmul(
                            t1,
                            E2[:, k],
                            Cb1[:, k],
                            start=(k == 0),
                            stop=(k == KC - 1),
                        )
                    if half == 1:
                        recipT = p_sm.tile(
                            [128, 1], fp32, tag="recipT", name=f"rT{b}"
                        )
                        nc.vector.reciprocal(recipT, t1[:, D : D + 1])
                        T1s = p_sm.tile([128, D], bf16, tag="t1s", name=f"T1s{b}")
                        nc.vector.tensor_scalar_mul(T1s, t1[:, 0:D], recipT)
                        T1ss.append(T1s)

                return [
                    lambda: ct_group(0),
                    lambda: (ct_group(1), cast_store()),
                    lambda: ct_group(2),
                    lambda: ct_group(3),
                    lambda: st_half(0),
                    lambda: s2_pair(0),
                    lambda: (st_half(1), s2_pair(1)),
                    lambda: (s2_pair(2), tile_a(0), tile_a(1)),
                    lambda: (s2_pair(3), t1_acc(0), tile_a(2), tile_a(3)),
                    lambda: (t1_acc(1), tile_a(4), tile_a(5)),
                    lambda: (tile_a(6), tile_a(7)),
                ]

            def tail_stages(b):
                """T1s-dependent part of batch b's tail: psB / CBm / CA."""
                E_q = E_qs[b]
                C1 = C1x[b]

                def tile_b(k):
                    T1s = T1ss[b]
                    kk = k % 4
                    osb = osbx[(b, k // 4)]
                    eq_k = E_q[:, k * 128 : (k + 1) * 128]
                    rr = rrx[(b, k)]
                    psB = ps_acc.tile(
                        [128, D + 2], fp32, tag="acc", name=f"psB{b}_{k}"
                    )
                    nc.tensor.matmul(psB[:, 0:D], eq_k, T1s, start=True, stop=True)
                    # CBm = (psB * rr) * C  (DVE fused)
                    nc.vector.scalar_tensor_tensor(
                        osb[:, kk, 2 * D : 3 * D], psB[:, 0:D], rr, C1[:, k],
                        MULT, MULT,
                    )
                    if kk == 3:
                        # CA = C * A for 4 tiles in one GPSIMD op
                        nc.gpsimd.tensor_mul(
                            osb[:, :, D : 2 * D],
                            C1[:, k - 3 : k + 1],
                            osb[:, :, 0:D],
                        )
                        if b == NB - 1:
                            # last batch: 2-tile stores on both rings to
                            # shrink the final DMA drain
                            for half in range(2):
                                eng = nc.sync if half == 0 else nc.scalar
                                k0 = k - 3 + 2 * half
                                eng.dma_start(
                                    out_d.ap()[
                                        b, k0 * 128 : (k0 + 2) * 128, D : 4 * D
                                    ].rearrange("(k p) n -> p k n", p=128),
                                    osb[:, 2 * half : 2 * half + 2],
                                )
                        else:
                            # alternate: Sync HWDGE ring / GPSIMD SWDGE ring
                            eng = (
                                nc.sync if (2 * b + k // 4) % 2 == 0 else nc.gpsimd
                            )
                            eng.dma_start(
                                out_d.ap()[
                                    b, (k - 3) * 128 : (k + 1) * 128, D : 4 * D
                                ].rearrange("(k p) n -> p k n", p=128),
                                osb,
                            )

                return [(lambda kk_: lambda: tile_b(kk_))(k) for k in range(KC)]

            # fine-grained software pipelining: interleave head(b) stages
            # with tail(b-1) stages so no engine queue head-of-line blocks
            for step in range(NB + 1):
                hs = head_stages(step) if step < NB else []
                ts = tail_stages(step - 1) if step >= 1 else []
                n = max(len(hs), len(ts))
                for i in range(n):
                    if i < len(hs):
                        hs[i]()
                    if i < len(ts):
                        ts[i]()

    nc.compile()
    return nc


def _get_nc():
    if "nc" not in _CACHE:
        _CACHE["nc"] = _build_nc()
    return _CACHE["nc"]


def _make_in_maps(C, Q, cmask, qmask, Wo_w):
    import ml_dtypes

    bf16 = ml_dtypes.bfloat16
    C = np.ascontiguousarray(C, dtype=np.float32)
    Q = np.ascontiguousarray(Q, dtype=np.float32)
    Wo_w = Wo_w.astype(np.float32)
    w1, w2, w3 = Wo_w[:D], Wo_w[D : 2 * D], Wo_w[2 * D :]

    NBND = KC * (D + 2) + 2 * 130 + (D + 2) + 1 + KC
    O_QT = KC * (D + 2)
    O_QB = O_QT + 2 * 130
    O_RQ = O_QB + (D + 2)
    O_CN = O_RQ + 1
    bnd = np.empty((B_FULL, 128, NBND), dtype=bf16)

    # Cb: tile layout with ones columns
    cb = bnd[:, :, 0:O_QT].reshape(B_FULL, 128, KC, D + 2)
    cb[:, :, :, 0:D] = C.reshape(B_FULL, KC, 128, D).transpose(0, 2, 1, 3)
    cb[:, :, :, D:] = 1.0

    # QT3w: [p, dk, j<128] = Q[b,j,dk*128+p]*w3[dk*128+p]; cols 128:130 = w1
    qt3 = bnd[:, :, O_QT:O_QB].reshape(B_FULL, 128, 2, 130)
    qt = Q.transpose(0, 2, 1).reshape(B_FULL, 2, 128, 128).transpose(0, 2, 1, 3)
    qt3[:, :, :, 0:128] = qt * w3.reshape(2, 128).T[None, :, :, None]
    qt3[:, :, :, 128:130] = w1.reshape(2, 128).T[None, :, :, None]

    # Qb with ones columns (rowsum source for psA)
    bnd[:, :, O_QB : O_QB + D] = Q
    bnd[:, :, O_QB + D : O_RQ] = 1.0

    # rq + qneg fused exp bias
    bnd[:, :, O_RQ] = Q @ w2 + (1.0 - qmask.astype(np.float32)) * NEG_INF

    # cneg in c-tile layout
    cneg = (1.0 - cmask.astype(np.float32)) * NEG_INF
    bnd[:, :, O_CN:] = cneg.reshape(B_FULL, KC, 128).transpose(0, 2, 1)

    in_maps = []
    for i in range(N_CORES):
        sl = slice(i * NB, (i + 1) * NB)
        in_maps.append({"bnd": np.ascontiguousarray(bnd[sl])})
    return in_maps


def kernel(C, Q, cmask, qmask, Wo_w, Wo_b):
    from concourse.bass_utils import run_bass_kernel_spmd

    nc = _get_nc()
    in_maps = _make_in_maps(C, Q, cmask, qmask, Wo_w)
    res = run_bass_kernel_spmd(nc, in_maps, core_ids=list(range(N_CORES)))
    out = np.concatenate([res.results[i]["out"] for i in range(N_CORES)], axis=0)
    return out
